# revision 23
# baseline (speedup 1.0000x reference)
"""Causal self-attention with RoPE on 8 Trainium2 NeuronCores.

Strategy (tensor-parallel over heads, SPMD-uniform, collective-free):
  - 12 heads -> 8 cores x 2 head slots (4 slots get zero weights).
  - Per core: QKV projection for its 2 heads in [channel, seq] layout in
    bf16 (x, weights and rope tables ship as bf16; qkv weights prescaled
    x64 on host with the descale folded into the exp scale and host
    w_proj); RoPE: only the "u" (natural-order) projection is done with
    weight matmuls - the 32-row-swapped "w" arrangement is produced by a
    single permutation matmul (P @ u) against a host-supplied 128x128
    permutation, halving the q/k projection matmul work; combines use
    sign-folded cos/sin tables on DVE; V computed directly in
    [seq, channel] layout with x as the stationary operand, stored TWICE:
    fp8e4 for the DoubleRow far-key path and bf16 for the diagonal path,
    both with a ones column at 64 (softmax denominators) padded to 96
    columns (DoubleRow stationary free/2 must be a multiple of 32).
  - Causal flash-style attention with scores kept transposed
    (S^T[keys, queries]), TILE-MAJOR across heads: each far key tile gets
    one 64-row score matmul PER HEAD back-to-back (head A in PE
    row-groups 0-1, head B in 2-3 via auto tile_position, so the two
    matmuls run concurrently in the array), one exp over 1024 columns
    (both heads) emitting fp8, and per 2-tile group one fp8 DoubleRow AV
    matmul per head contracting 256 keys (far keys average over >=512
    p-weights, so fp8 noise washes out); the 4 diagonal tiles - whose
    few-key queries dominate the error - use a bf16 path with a pre-exp
    additive causal mask (-1e9), also exp'd once for both heads.
  - Per-head normalization via the ones-column denominators; partial
    output projection (fp32r) through the core's slice of w_proj
    columns; bf16 partial outputs. Host sums the 8 partials in f64.
  - Schedule: instead of lumpy per-phase emission, chunk J+1's QKV and
    chunk J-1's output projection are split into small units (one
    matmul-group each) and woven evenly between chunk J's attention
    items, so the exp stream on ACT (the pacing engine) never starves
    and PE fills its exp-wait bubbles with projection work.  AV matmuls
    are software-pipelined ONE ITEM LATE: real PE executes MATMULs
    strictly in order (only LDWEIGHTS reorders), so an AV emitted in
    place would sit blocked on its exp and hold up the next item's
    score matmuls, gapping ACT every group.
"""

import os
import sys

sys.path.insert(0, "/opt/trn_rl_repo")
# recover gracefully if a previous session left the NeuronCores wedged
os.environ.setdefault("NEURON_RT_RESET_CORES", "1")

import numpy as np

import concourse.bass as bass
import concourse.mybir as mybir
import concourse.tile as tile
from concourse import bacc, bass_utils
from concourse.masks import make_identity

FP32 = mybir.dt.float32
FP32R = mybir.dt.float32r
FP8 = mybir.dt.float8e4
BF16 = mybir.dt.bfloat16
AF = mybir.ActivationFunctionType
ALU = mybir.AluOpType
DR = mybir.MatmulPerfMode.DoubleRow

T = 4096
C = 768
D = 64
N_HEAD = 12
N_CORES = 8
CHUNK = 512          # query chunk (matmul free dim)
NCHUNK = T // CHUNK  # 8
KT = 128             # key tile
ROPE_BASE = 10000.0
W_SCALE = 64.0       # host prescale on fp8 qkv weights (subnormal avoidance)
EXP_SCALE = 0.125 / (W_SCALE * W_SCALE)  # = 2**-15, folds 1/sqrt(D) + descale

# core -> (head_slot_a, head_slot_b); None = zero slot
HEAD_MAP = [(0, 8), (1, 9), (2, 10), (3, 11),
            (4, None), (5, None), (6, None), (7, None)]

_PROG = None  # cached compiled program


def build_program(reps=1, staggered=False):
    """Build + compile the per-core Bass program (identical on all cores).

    ``reps > 1`` wraps the entire kernel body in a hardware loop executing
    it ``reps`` times back-to-back - used by the timing harness to amortize
    the (multi-ms) axon-tunnel dispatch overhead over many on-device
    executions so the per-execution slope isolates true HW time.  Every
    iteration performs the complete kernel, including all input DMA."""
    import contextlib

    nc = bacc.Bacc("TRN2", target_bir_lowering=False, debug=False,
                   num_devices=N_CORES)

    xT_d = nc.dram_tensor("xT", [C, T], BF16, kind="ExternalInput").ap()
    wqk_u_d = nc.dram_tensor("wqk_u", [C, 256], BF16, kind="ExternalInput").ap()
    w_v_d = nc.dram_tensor("w_v", [C, 128], BF16, kind="ExternalInput").ap()
    w_pT_d = nc.dram_tensor("w_projT", [128, C], FP32R, kind="ExternalInput").ap()
    cos_d = nc.dram_tensor("rope_cos", [128, T], BF16, kind="ExternalInput").ap()
    sin_d = nc.dram_tensor("rope_sin", [128, T], BF16, kind="ExternalInput").ap()
    mask_d = nc.dram_tensor("maskneg", [128, KT], FP32, kind="ExternalInput").ap()
    perm_d = nc.dram_tensor("perm", [128, 128], BF16, kind="ExternalInput").ap()
    out_d = nc.dram_tensor("outT", [C, T], BF16, kind="ExternalOutput").ap()

    with tile.TileContext(nc) as tc:
        with (
            tc.tile_pool(name="persist", bufs=1) as pers,
            tc.tile_pool(name="xin", bufs=3) as xin,
            tc.tile_pool(name="tmp", bufs=4) as tmps,
            tc.tile_pool(name="ptile", bufs=8) as ptile,
            tc.tile_pool(name="ostage", bufs=4) as ostage,
            tc.tile_pool(name="small", bufs=4) as small,
            tc.tile_pool(name="psUW", bufs=2, space="PSUM") as psUW,  # u/w/v accums
            tc.tile_pool(name="psS", bufs=2, space="PSUM") as psS,    # score pairs + proj
            tc.tile_pool(name="psY", bufs=2, space="PSUM") as psY,    # y accum
            tc.For_i(0, reps, staggered_reset=staggered)
            if reps > 1 else contextlib.nullcontext(),
        ):
            # ---- persistent SBUF ----
            wqk_u = pers.tile([128, 6, 256], BF16)
            w_v = pers.tile([128, 6, 128], BF16)
            w_pT = pers.tile([128, C], FP32R)
            perm_sb = pers.tile([128, 128], BF16)

            mask_sb = pers.tile([128, KT], FP32)
            QT = pers.tile([128, T], BF16)   # rows 0-63 head A, 64-127 head B
            KTt = pers.tile([128, T], BF16)
            V = pers.tile([128, 32, 2, 96], FP8)   # [key%128, keytile, head, v|1|pad]
            Vb = pers.tile([128, 32, 2, 96], BF16)  # bf16 twin for diagonal AV
            # DoubleRow stationary free/2 must be a multiple of 32 -> pad
            # each head's block to 96 (cols 65:96 zeroed, never normalized)
            Y = pers.tile([128, T], FP32R)    # normalized attention out [ych, q]
            ones_sb = pers.tile([128, D], FP8)
            ones_bf = pers.tile([128, D], BF16)
            ident = pers.tile([128, 128], FP32)

            # issue all initial DMAs before any engine work queues up
            nc.sync.dma_start(wqk_u[:], wqk_u_d.rearrange("(o p) m -> p o m", p=128))
            nc.gpsimd.dma_start(perm_sb[:], perm_d[:])
            nc.gpsimd.dma_start(mask_sb[:], mask_d[:])
            nc.gpsimd.dma_start(w_v[:], w_v_d.rearrange("(o p) m -> p o m", p=128))
            nc.gpsimd.dma_start(w_pT[:], w_pT_d[:])
            ones_f32 = pers.tile([128, D], FP32)
            nc.any.memset(ones_f32[:], 1.0)
            # preload the Exp activation table off the critical path
            warm_act = small.tile([1, 8], FP32, tag="wact")
            nc.scalar.activation(warm_act[:], ones_f32[0:1, 0:8], AF.Exp)
            make_identity(nc, ident[:])
            # HAM warm-up: keep PE busy during the initial input DMAs so the
            # clock gate reaches 8/8 before the first real matmuls (results
            # discarded; the tiny copy keeps DCE from dropping the chain)
            warm_ps = psS.tile([128, 2, CHUNK], FP32, tag="s")
            for i in range(8):
                nc.tensor.matmul(warm_ps[0:64, 0, 0:64], ident[:, 0:64],
                                 ident[:, 0:64], start=True, stop=True)
            warm_sb = small.tile([1, 8], FP32, tag="warm")
            nc.vector.tensor_copy(warm_sb[:], warm_ps[0:1, 0, 0:8])
            nc.vector.tensor_copy(ones_sb[:], ones_f32[:])
            nc.vector.tensor_copy(ones_bf[:], ones_f32[:])
            nc.vector.tensor_copy(V[:, :, 0, 64], ones_sb[:, 0:32])
            nc.vector.tensor_copy(V[:, :, 1, 64], ones_sb[:, 0:32])
            nc.vector.tensor_copy(Vb[:, :, 0, 64], ones_bf[:, 0:32])
            nc.vector.tensor_copy(Vb[:, :, 1, 64], ones_bf[:, 0:32])
            nc.gpsimd.memset(V[:, :, :, 65:96], 0.0)
            nc.gpsimd.memset(Vb[:, :, :, 65:96], 0.0)

            def qkv_units(J):
                """Chunk J's QKV projection as a list of small emission
                units to weave between attention items of chunk J-1."""
                cols = slice(J * CHUNK, (J + 1) * CHUNK)
                st = {}

                def u_dma():
                    xt = xin.tile([128, 6, CHUNK], BF16, name="xt")
                    st["xt"] = xt
                    xT_r = xT_d.rearrange("(o p) n -> p o n", p=128)
                    nc.sync.dma_start(xt[:, 0:3, :], xT_r[:, 0:3, cols])
                    nc.sync.dma_start(xt[:, 3:6, :], xT_r[:, 3:6, cols])
                    cs_sb = tmps.tile([128, CHUNK], BF16, tag="cs")
                    sn_sb = tmps.tile([128, CHUNK], BF16, tag="sn")
                    st["cs"] = cs_sb
                    st["sn"] = sn_sb
                    # J=0's tables are on the critical startup path: the
                    # scalar HWDGE queue is idle then; later issues ride
                    # gpsimd
                    dma_eng = nc.scalar if J == 0 else nc.gpsimd
                    dma_eng.dma_start(cs_sb[:], cos_d[:, cols])
                    dma_eng.dma_start(sn_sb[:], sin_d[:, cols])

                def mk_u(qk, half):
                    # split the 6-matmul accumulation into two 3-matmul
                    # units: a unit's matmuls occupy the PE FIFO ahead of
                    # the next score matmuls, so unit length bounds how
                    # long the ACT exp stream can starve
                    def f():
                        if half == 0:
                            st[("u", qk)] = psUW.tile([128, CHUNK], FP32,
                                                      tag="uw", name="u_ps")
                        u_ps = st[("u", qk)]
                        wcol = slice(qk * 128, qk * 128 + 128)
                        for k in (0, 1, 2) if half == 0 else (3, 4, 5):
                            nc.tensor.matmul(u_ps[:], wqk_u[:, k, wcol],
                                             st["xt"][:, k, :],
                                             start=(k == 0), stop=(k == 5))
                        if half == 1:
                            u_sb = tmps.tile([128, CHUNK], BF16, tag="usb",
                                             name="u_sb")
                            nc.vector.tensor_copy(u_sb[:], u_ps[:])
                            st[("usb", qk)] = u_sb
                    return f

                def mk_w(qk):
                    def f():
                        w_ps = psUW.tile([128, CHUNK], FP32, tag="uw",
                                         name="w_ps")
                        u_sb = st[("usb", qk)]
                        # 32-row-half swap within each 64 block via one
                        # permutation matmul (replaces a 2nd 6-matmul
                        # projection against swapped weights)
                        nc.tensor.matmul(w_ps[:], perm_sb[:], u_sb[:],
                                         start=True, stop=True)
                        tgt = QT if qk == 0 else KTt
                        tm = tmps.tile([128, CHUNK], BF16, tag="ropetmp",
                                       name="tm")
                        nc.vector.tensor_tensor(tgt[:, cols], u_sb[:],
                                                st["cs"][:], ALU.mult)
                        nc.vector.tensor_tensor(tm[:], w_ps[:], st["sn"][:],
                                                ALU.mult)
                        nc.vector.tensor_tensor(tgt[:, cols], tgt[:, cols],
                                                tm[:], ALU.add)
                    return f

                def mk_v(s):
                    def f():
                        # v directly in [seq, ch] layout: x slice stationary
                        if s == 0:
                            st["v"] = psUW.tile([128, CHUNK], FP32,
                                                tag="uw", name="v_ps")
                        v_ps = st["v"]
                        ssl = slice(s * 128, (s + 1) * 128)
                        for k in range(6):
                            nc.tensor.matmul(v_ps[:, ssl],
                                             st["xt"][:, k, ssl],
                                             w_v[:, k, :],
                                             start=(k == 0), stop=(k == 5))
                        if s == 3:
                            # one copy per twin drops the 4x[seq128, vA|vB]
                            # quarters into [keytile, head, 96], skipping
                            # the ones/pad columns
                            nc.vector.tensor_copy(
                                V[:, 4 * J:4 * J + 4, :, 0:64], v_ps[:])
                            nc.vector.tensor_copy(
                                Vb[:, 4 * J:4 * J + 4, :, 0:64], v_ps[:])
                    return f

                return [u_dma, mk_u(0, 0), mk_u(0, 1), mk_w(0),
                        mk_u(1, 0), mk_u(1, 1), mk_w(1),
                        mk_v(0), mk_v(1), mk_v(2), mk_v(3)]

            def proj_unit(J, m, on_act=False):
                # output projection m-tile for chunk J through the psUW
                # ring (1-bank tiles), keeping the psS ring free for the
                # score/exp ping-pong; the DVE copy overlaps pair exps
                def f():
                    cols = slice(J * CHUNK, (J + 1) * CHUNK)
                    o_ps = psUW.tile([128, CHUNK], FP32, tag="uw",
                                     name="o_ps")
                    nc.tensor.matmul(o_ps[:],
                                     w_pT[:, m * 128:(m + 1) * 128],
                                     Y[:, cols], start=True, stop=True)
                    o_sb = ostage.tile([128, CHUNK], BF16, tag="osb",
                                       name="o_sb")
                    if on_act:
                        nc.scalar.copy(o_sb[:], o_ps[:])
                    else:
                        nc.vector.tensor_copy(o_sb[:], o_ps[:])
                    nc.sync.dma_start(
                        out_d.rearrange("(a p) n -> p a n", p=128)[:, m, cols],
                        o_sb[:])
                return f

            def emit_pair_group(J, k, ys, first, last):
                # one group = 2 far key tiles x BOTH heads: the per-head
                # score matmuls contract only 64 rows, so head A rides PE
                # row-groups 0-1 and head B row-groups 2-3 (tile_position
                # auto-derived from base partitions) - adjacent emission
                # lets them run CONCURRENTLY in the array; one exp per key
                # tile covers both heads; one DoubleRow AV per head.
                # Returns the AV emission as a closure: the caller defers
                # it one item because real PE MATMULs are strict FIFO - an
                # AV emitted here would block the NEXT item's score
                # matmuls until this item's exp drains, gapping ACT
                cols = slice(J * CHUNK, (J + 1) * CHUNK)
                p4 = ptile.tile([128, 2, 2, CHUNK], FP8, tag="p")
                for u in range(2):
                    t = 2 * k + u
                    s2 = psS.tile([128, 2, CHUNK], FP32, tag="s")
                    for h in range(2):
                        hsl = slice(64 * h, 64 * h + 64)
                        nc.tensor.matmul(
                            s2[:, h, :], KTt[hsl, t * KT:(t + 1) * KT],
                            QT[hsl, cols], start=True, stop=True)
                    nc.scalar.activation(p4[:, u, :, :], s2[:], AF.Exp,
                                         scale=EXP_SCALE)

                def av():
                    for h in range(2):
                        nc.tensor.matmul(
                            ys[h][:], V[:, 2 * k:2 * k + 2, h, :],
                            p4[:, :, h, :],
                            start=first, stop=last, perf_mode=DR)
                return av

            def emit_diag(J, d, ys, first, last):
                # diagonal (masked) key tile for BOTH heads: two 64-row
                # score matmuls (concurrent row-groups), two mask adds,
                # ONE exp over both heads, two bf16 AV matmuls (returned
                # as a deferred closure, same as emit_pair_group)
                t = 4 * J + d
                qlo = 128 * d
                sub = slice(qlo, CHUNK)
                qsub = slice(J * CHUNK + qlo, (J + 1) * CHUNK)
                s1 = psS.tile([128, 2, CHUNK], FP32, tag="s")
                for h in range(2):
                    hsl = slice(64 * h, 64 * h + 64)
                    nc.tensor.matmul(
                        s1[:, h, sub], KTt[hsl, t * KT:(t + 1) * KT],
                        QT[hsl, qsub], start=True, stop=True)
                    nc.vector.tensor_tensor(
                        s1[:, h, qlo:qlo + KT], s1[:, h, qlo:qlo + KT],
                        mask_sb[:], ALU.add)
                p1 = ptile.tile([128, 2, CHUNK], BF16, tag="p1")
                nc.scalar.activation(p1[:, :, sub], s1[:, :, sub], AF.Exp,
                                     scale=EXP_SCALE)

                def av():
                    for h in range(2):
                        nc.tensor.matmul(
                            ys[h][:, sub], Vb[:, t, h, :], p1[:, h, sub],
                            start=first, stop=last)
                return av

            def emit_norm(J, h, y_ps):
                cols = slice(J * CHUNK, (J + 1) * CHUNK)
                hsl = slice(64 * h, 64 * h + 64)
                rc = small.tile([1, CHUNK], FP32R, tag="rc")
                with nc.allow_low_precision(reason="f32r recip for softmax denom"):
                    nc.vector.reciprocal(rc[0:1, :], y_ps[64:65, :])
                rb = small.tile([64, CHUNK], FP32R, tag="rb")
                nc.gpsimd.partition_broadcast(rb[:], rc[0:1, :])
                nc.vector.tensor_tensor(Y[hsl, cols], y_ps[0:64, :], rb[:], ALU.mult)

            def att_items(J):
                # interleave the 4 diagonal (masked) chains among the pair
                # groups so group exps fill the S->mask->exp latency
                # bubbles; keep the first two and last slots group-only
                groups = [("p", k) for k in range(2 * J)]
                diags = [("d", d) for d in range(4)]
                if 2 * J >= 7:
                    items = groups[:2]
                    rest = groups[2:]
                    for i, dd in enumerate(diags):
                        items.append(dd)
                        if i < len(rest):
                            items.append(rest[i])
                    items.extend(rest[4:])
                else:
                    items = diags + groups
                return items

            def emit_item(J, it, ys, first, last):
                if it[0] == "p":
                    return emit_pair_group(J, it[1], ys, first, last)
                return emit_diag(J, it[1], ys, first, last)

            # Emission per chunk J (steady state): every attention item
            # covers BOTH heads (their y chains accumulate in the two psY
            # banks simultaneously); chunk J+1's QKV units and chunk J-1's
            # projection units are inserted at even spacing into the item
            # stream so PE's exp-wait bubbles absorb them and ACT never
            # starves.
            for u in qkv_units(0):
                u()
            for J in range(NCHUNK):
                items = att_items(J)
                n_it = len(items)
                units = []
                if J + 1 < NCHUNK:
                    units.extend(qkv_units(J + 1))
                if J > 0:
                    units.extend(proj_unit(J - 1, m) for m in range(6))

                # stream entries: ("i", idx, it) | ("n", h) | ("u", fn)
                stream = [("i", i, it) for i, it in enumerate(items)]
                stream.append(("n", 0))
                stream.append(("n", 1))

                if units:
                    # weave: skip the first 2 entries (let the exp stream
                    # start), then insert a unit every `step` entries so
                    # all units land before ~the last entry
                    n_s = len(stream)
                    step = max(1, (n_s - 2) // len(units))
                    woven = []
                    ui = 0
                    for idx, e in enumerate(stream):
                        woven.append(e)
                        if idx >= 1 and (idx - 1) % step == 0 and ui < len(units):
                            woven.append(("u", units[ui]))
                            ui += 1
                    # any leftovers (short streams): append at end
                    for u in units[ui:]:
                        woven.append(("u", u))
                    stream = woven

                ys = [psY.tile([96, CHUNK], FP32, tag="y", name="y0"),
                      psY.tile([96, CHUNK], FP32, tag="y", name="y1")]
                # software-pipeline the AV matmuls one item late: PE is
                # strict FIFO for MATMULs, so an in-place AV (waiting on
                # its exp) would hold up the next item's score matmuls and
                # starve ACT; deferred one item, the exp has ~a full
                # group's slack to drain before PE reaches the AV
                pending_av = None
                for e in stream:
                    if e[0] == "i":
                        _, i, it = e
                        av = emit_item(J, it, ys, i == 0, i == n_it - 1)
                        if pending_av is not None:
                            pending_av()
                        pending_av = av
                    elif e[0] == "n":
                        if pending_av is not None:
                            pending_av()
                            pending_av = None
                        emit_norm(J, e[1], ys[e[1]])
                    else:
                        e[1]()
            for m in range(6):
                proj_unit(NCHUNK - 1, m, on_act=True)()

    nc.compile()
    return nc


def _rope_tables():
    theta = 1.0 / (ROPE_BASE ** (np.arange(0, D, 2, dtype=np.float32) / D))  # [32]
    freqs = np.arange(T, dtype=np.float32)[None, :] * theta[:, None]  # [32, T]
    bf = mybir.dt.np(BF16)
    cos32 = np.cos(freqs).astype(np.float32)
    sin32 = np.sin(freqs).astype(np.float32)
    cos128 = np.tile(cos32, (4, 1)).astype(bf)
    sin128 = np.concatenate([-sin32, sin32, -sin32, sin32], axis=0).astype(bf)
    return cos128, sin128


def _masks():
    # additive causal mask for the diagonal 128-strip: key row kk may only
    # attend query column j >= kk (strip-local coords are d-independent)
    kk = np.arange(128)[:, None]
    jj = np.arange(KT)[None, :]
    return np.where(kk > jj, np.float32(-1e9), np.float32(0.0))


def _perm128():
    # P[i, swap(i)] = 1 where swap exchanges 32-row halves within each
    # 64-row block (P is symmetric, P == P.T)
    i = np.arange(128)
    j = np.where(i % 64 < 32, i + 32, i - 32)
    P = np.zeros((128, 128), np.float32)
    P[i, j] = 1.0
    return P.astype(mybir.dt.np(BF16))


def make_in_maps(x, w_attn, w_proj):
    bf = mybir.dt.np(BF16)
    xT = np.ascontiguousarray(x.reshape(T, C).T).astype(bf)  # [C, T]
    cos128, sin128 = _rope_tables()
    masks = _masks()
    perm = _perm128()
    in_maps = []
    for c in range(N_CORES):
        qk_rows = []   # rows of w_attn for [qA, qB, kA, kB]
        v_rows = []    # [vA, vB]
        p_cols = []    # w_proj columns for [A(64), B(64)]
        sel = HEAD_MAP[c]
        for part_base in (0, C):  # q rows then k rows
            for h in sel:
                if h is None:
                    qk_rows.append(np.zeros((64, C), np.float32))
                else:
                    qk_rows.append(w_attn[part_base + 64 * h: part_base + 64 * h + 64])
        for h in sel:
            if h is None:
                v_rows.append(np.zeros((64, C), np.float32))
                p_cols.append(np.zeros((C, 64), np.float32))
            else:
                v_rows.append(w_attn[2 * C + 64 * h: 2 * C + 64 * h + 64])
                p_cols.append(w_proj[:, 64 * h: 64 * h + 64])
        qk = np.concatenate(qk_rows, axis=0) * W_SCALE      # [256, C]
        wqk_u = np.ascontiguousarray(qk.T).astype(bf)       # [C, 256]
        w_v = np.ascontiguousarray(
            (np.concatenate(v_rows, axis=0) * W_SCALE).T).astype(bf)  # [C, 128]
        w_pT = np.ascontiguousarray(
            np.concatenate(p_cols, axis=1).T / W_SCALE)     # [128, C]
        in_maps.append({
            "xT": xT, "wqk_u": wqk_u, "w_v": w_v,
            "w_projT": w_pT.astype(np.float32), "rope_cos": cos128,
            "rope_sin": sin128, "maskneg": masks, "perm": perm,
        })
    return in_maps


def kernel(x, w_attn, w_proj):
    global _PROG
    x = np.asarray(x, dtype=np.float32)
    w_attn = np.asarray(w_attn, dtype=np.float32)
    w_proj = np.asarray(w_proj, dtype=np.float32)
    if _PROG is None:
        _PROG = build_program()
    nc = _PROG
    in_maps = make_in_maps(x, w_attn, w_proj)
    res = bass_utils.run_bass_kernel_spmd(nc, in_maps, core_ids=list(range(N_CORES)))
    acc = np.zeros((C, T), dtype=np.float64)
    for c in range(N_CORES):
        acc += res.results[c]["outT"].astype(np.float64)
    return np.ascontiguousarray(acc.T.astype(np.float32)).reshape(1, T, C)


if __name__ == "__main__":
    rng = np.random.default_rng(0)
    x = rng.standard_normal((1, T, C)).astype(np.float32)
    wa = (rng.standard_normal((3 * C, C)) * 0.02).astype(np.float32)
    wp = (rng.standard_normal((C, C)) * 0.02).astype(np.float32)
    y = kernel(x, wa, wp)
    print("kernel out", y.shape, y.dtype, float(np.abs(y).max()))
